# Initial kernel scaffold
#
"""Talking-heads attention (ViT-B/16-ish shapes) on 8 Trainium2 NeuronCores.

Problem: B=16, N=577, C=768, H=12 heads, d=64.
  qkv = x @ Wqkv.T ; logits = q k^T * scale ; pre-softmax head mix (Wpre);
  softmax ; post-softmax head mix (Wpost) ; out = (attn @ v) @ Wproj.T + b.

Distribution: pure data-parallel over batch, 2 batches per core, no
collectives.

Per-core design (all matmuls bf16 inputs, fp32 PSUM accumulation):
  - host pre-transposes x to [C, N] and pre-casts/packs all weights.
  - qkv:   q,k in [feat, tok] layout; v in [tok, feat] layout.
  - logits per head, K=64, two heads run concurrently via PE row groups.
  - talking-heads mixing runs as 120x120 block-diagonal matmuls in a packed
    layout [(h-major: p = 10h + n_i), m] over blocks of 10 query rows.
    The partition interleave that builds this layout is impossible in one
    SBUF->SBUF DMA (only the first AP dim may cross partitions), so the pack
    round-trips through a DRAM scratch laid out [p][b][m] per qtile:
    10 stride-10-partition write DMAs + 3 contiguous read DMAs. The read
    is detached from the logits stage (issued a full iteration later), so
    the DRAM bounce decouples producer and consumer completely.
  - softmax without max-subtraction (logits are small); exp on ScalarE with
    accum_out producing the row sums. The exp stream (~0.92us per block) is
    the slowest per-block stage, so the PE emission interleaves premix
    blocks with logits head-pairs and postmix groups of the two adjacent
    pipeline stages -- the in-order PE always has work while PSUM slots
    wait on exps.
  - normalization is folded into the postmix: rows of the moving operand
    bdpostT are scaled by 1/S per block (12 ops of [120,120] instead of
    [120,577]); exact since postmix contracts over p=(g,ni) and S is per-p.
  - post-mix is fused with the transpose AV needs: E-tile stationary,
    scaled block-diag Wpost^T moving, giving P'^T[m, (10g+n)] in PSUM.
  - AV consumes P'^T with a strided free AP per head; head pairs run
    concurrently via PE column groups; output feeds proj with no transpose.
  - Steady-state iteration `it` emits, interleaved per 4-block group:
      premix+exp(it+1) | logits(it+3) | postmix(it); then AV+proj(it),
      pack-writes(it+3); pack-read(it+2) opens the iteration.
"""

import numpy as np
import ml_dtypes

import concourse.bass as bass
import concourse.mybir as mybir
from concourse import bacc
from concourse.tile import TileContext
from concourse.bass_utils import run_bass_kernel_spmd

BF16 = ml_dtypes.bfloat16

B, N, C, H = 16, 577, 768, 12
D = C // H                 # 64
NCORES = 8
BPC = B // NCORES          # batches per core = 2
NPAD = 600                 # padded query-token count (5 qtiles of 120)
QT = 5                     # query tiles
QTW = 120                  # rows per query tile
NI = 10                    # query rows per packed block
BPQ = QTW // NI            # blocks per qtile = 12
FT = C // 128              # feature tiles = 6
MT = [128, 128, 128, 128, 65]   # key-token tiles (sum 577)
MOF = [0, 128, 256, 384, 512]

_NC_CACHE = {}


def _build_nc(debug=False):
    nc = bacc.Bacc("TRN2", target_bir_lowering=False)
    dt = mybir.dt

    xT = nc.dram_tensor("xT", [BPC, C, NPAD], dt.bfloat16, kind="ExternalInput")
    wqT = nc.dram_tensor("wqT", [C, C], dt.bfloat16, kind="ExternalInput")
    wkT = nc.dram_tensor("wkT", [C, C], dt.bfloat16, kind="ExternalInput")
    wvT = nc.dram_tensor("wvT", [C, C], dt.bfloat16, kind="ExternalInput")
    wpT = nc.dram_tensor("wpT", [C, C], dt.bfloat16, kind="ExternalInput")
    bdpre = nc.dram_tensor("bdpre", [QTW, QTW], dt.bfloat16, kind="ExternalInput")
    bdpostT = nc.dram_tensor("bdpostT", [QTW, QTW], dt.bfloat16, kind="ExternalInput")
    bias = nc.dram_tensor("bias", [C], dt.float32, kind="ExternalInput")
    y = nc.dram_tensor("y", [BPC, N, C], dt.float32, kind="ExternalOutput")
    # logits bounce scratch in natural order [batch][qtile][nq][h][m];
    # the packed interleave happens on the strided read side
    pk = nc.dram_tensor("pk", [BPC, QT, QTW, H, N], dt.bfloat16, kind="Internal")

    with TileContext(nc) as tc:
        with (
            tc.tile_pool(name="consts", bufs=1) as consts,
            tc.tile_pool(name="xp", bufs=1) as xp,
            tc.tile_pool(name="qkv", bufs=2) as qkvp,
            tc.tile_pool(name="vpool", bufs=1) as vpool,
            tc.tile_pool(name="opool", bufs=2) as opool,
            tc.tile_pool(name="lnatp", bufs=2) as lnatp,
            tc.tile_pool(name="lpkp", bufs=2) as lpkp,
            tc.tile_pool(name="ep", bufs=2) as ep,
            tc.tile_pool(name="ptp", bufs=1) as ptp,
            tc.tile_pool(name="bdp", bufs=2) as bdp,
            tc.tile_pool(name="stage", bufs=2) as stage,
            tc.tile_pool(name="outp", bufs=2) as outp,
            tc.tile_pool(name="ps_a", bufs=2, space="PSUM") as ps_a,
            tc.tile_pool(name="ps_b", bufs=4, space="PSUM") as ps_b,
        ):
            # ---- constants (split per kc-column so qkv starts early) ----
            wq_sb = consts.tile([128, FT, C], dt.bfloat16, tag="wq")
            wk_sb = consts.tile([128, FT, C], dt.bfloat16, tag="wk")
            wv_sb = consts.tile([128, FT, C], dt.bfloat16, tag="wv")
            wp_sb = consts.tile([128, FT, C], dt.bfloat16, tag="wp")
            for w_sb, w_dr, eng in ((wq_sb, wqT, nc.scalar), (wk_sb, wkT, nc.gpsimd),
                                    (wv_sb, wvT, nc.scalar), (wp_sb, wpT, nc.gpsimd)):
                eng.dma_start(out=w_sb[:], in_=w_dr.rearrange("(t p) f -> p t f", p=128))
            bdpre_sb = consts.tile([QTW, QTW], dt.bfloat16, tag="bdpre")
            nc.scalar.dma_start(out=bdpre_sb[:], in_=bdpre[:])
            bdpostT_sb = consts.tile([QTW, QTW], dt.bfloat16, tag="bdpostT")
            nc.gpsimd.dma_start(out=bdpostT_sb[:], in_=bdpostT[:])
            bias_sb = consts.tile([128, C], dt.float32, tag="bias")
            nc.scalar.dma_start(
                out=bias_sb[:],
                in_=bass.AP(tensor=bias[:].tensor, offset=0, ap=[[0, 128], [1, C]]),
            )

            def make_batch(bj):
                """Per-batch pipeline context; all emitters bound to batch bj."""
                ctx = {}
                lnats, lpks, softs = {}, {}, {}

                xT_sb = xp.tile([128, FT, NPAD], dt.bfloat16, tag="xT", name="xT_sb")
                nc.sync.dma_start(
                    out=xT_sb[:], in_=xT[bj].rearrange("(t p) n -> p t n", p=128)
                )
                q_sb = qkvp.tile([128, FT, NPAD], dt.bfloat16, tag="q", name="q_sb")
                k_sb = qkvp.tile([128, FT, N], dt.bfloat16, tag="k", name="k_sb")
                v_sb = vpool.tile([128, len(MT), C], dt.bfloat16, tag="v", name="v_sb")
                ctx["v_sb"] = v_sb

                def qk_chunk(ft, dst, w_sb, ntok):
                    ps = ps_a.tile([128, NPAD], dt.float32, tag="a", name="ps")
                    for kc in range(FT):
                        for lo, hi in ((0, 512), (512, ntok)):
                            nc.tensor.matmul(
                                out=ps[:, lo:hi],
                                lhsT=w_sb[:, kc, ft * 128:(ft + 1) * 128],
                                rhs=xT_sb[:, kc, lo:hi],
                                start=(kc == 0), stop=(kc == FT - 1),
                            )
                    if ft % 2 == 0:
                        nc.vector.tensor_copy(out=dst[:, ft, :], in_=ps[:, 0:ntok])
                    else:
                        nc.scalar.copy(out=dst[:, ft, :], in_=ps[:, 0:ntok])

                def v_chunk(mt):
                    ps = ps_a.tile([128, C], dt.float32, tag="a", name="ps")
                    mw = MT[mt]
                    for kc in range(FT):
                        for lo, hi in ((0, 512), (512, C)):
                            nc.tensor.matmul(
                                out=ps[0:mw, lo:hi],
                                lhsT=xT_sb[:, kc, MOF[mt]:MOF[mt] + mw],
                                rhs=wv_sb[:, kc, lo:hi],
                                start=(kc == 0), stop=(kc == FT - 1),
                            )
                    if mt % 2 == 0:
                        nc.vector.tensor_copy(out=v_sb[0:mw, mt, :], in_=ps[0:mw, 0:C])
                    else:
                        nc.scalar.copy(out=v_sb[0:mw, mt, :], in_=ps[0:mw, 0:C])

                def new_lnat(qt):
                    lnats[qt] = lnatp.tile([QTW, H, N], dt.bfloat16, tag="lnat",
                                           name="l_nat")

                def logits_pair(qt, hp):
                    q0 = qt * QTW
                    l_nat = lnats[qt]
                    ps0 = ps_a.tile([QTW, N], dt.float32, tag="a", name="ps0")
                    ps1 = ps_a.tile([QTW, N], dt.float32, tag="a", name="ps1")
                    for sub, ps in ((0, ps0), (1, ps1)):
                        pbase = 64 * sub
                        for lo, hi in ((0, 512), (512, N)):
                            nc.tensor.matmul(
                                out=ps[:, lo:hi],
                                lhsT=q_sb[pbase:pbase + 64, hp, q0:q0 + QTW],
                                rhs=k_sb[pbase:pbase + 64, hp, lo:hi],
                            )
                    if hp in (2, 5):
                        nc.scalar.copy(out=l_nat[:, 2 * hp, :], in_=ps0[:])
                    else:
                        nc.vector.tensor_copy(out=l_nat[:, 2 * hp, :], in_=ps0[:])
                    nc.vector.tensor_copy(out=l_nat[:, 2 * hp + 1, :], in_=ps1[:])

                def pack_writes(qt):
                    nc.sync.dma_start(out=pk[bj, qt], in_=lnats.pop(qt)[:])

                def pack_read(qt):
                    l_pk = lpkp.tile([QTW, BPQ, N], dt.bfloat16, tag="lpk",
                                     name="l_pk")
                    ov = l_pk[:].rearrange("(h n) b m -> n h b m", n=NI)
                    iv = pk[bj, qt].rearrange("(b n) h m -> n h b m", n=NI)
                    for ni in range(NI):
                        nc.gpsimd.dma_start(out=ov[ni], in_=iv[ni])
                    lpks[qt] = l_pk

                def new_soft(qt):
                    softs[qt] = (
                        ep.tile([QTW, BPQ, N], dt.bfloat16, tag="e", name="e_sb"),
                        stage.tile([QTW, BPQ], dt.float32, tag="s", name="s_sb"),
                        stage.tile([QTW, BPQ], dt.float32, tag="sinv", name="sinv"),
                        bdp.tile([QTW, BPQ, QTW], dt.bfloat16, tag="bdsc",
                                 name="bd_sc"),
                    )

                def soft_block(qt, b):
                    e_sb, s_sb, sinv, bd_sc = softs[qt]
                    l_pk = lpks[qt]
                    ps = ps_a.tile([QTW, N], dt.float32, tag="a", name="ps")
                    for lo, hi in ((0, 512), (512, N)):
                        nc.tensor.matmul(
                            out=ps[:, lo:hi], lhsT=bdpre_sb[:], rhs=l_pk[:, b, lo:hi]
                        )
                    nc.scalar.activation(
                        out=e_sb[:, b, :], in_=ps[:],
                        func=mybir.ActivationFunctionType.Exp,
                        accum_out=s_sb[:, b:b + 1],
                    )
                    nc.vector.reciprocal(out=sinv[:, b:b + 1], in_=s_sb[:, b:b + 1])
                    nc.vector.tensor_scalar_mul(
                        bd_sc[:, b, :], bdpostT_sb[:], sinv[:, b:b + 1]
                    )

                def postmix_group(qt, pt_sb, bg, mts):
                    e_sb, _, _, bd_sc = softs[qt]
                    for mt in mts:
                        mw = MT[mt]
                        ps = ps_b.tile([128, 4 * QTW], dt.float32, tag="b", name="ps")
                        for sl in range(4):
                            b = 4 * bg + sl
                            nc.tensor.matmul(
                                out=ps[0:mw, sl * QTW:(sl + 1) * QTW],
                                lhsT=e_sb[:, b, MOF[mt]:MOF[mt] + mw],
                                rhs=bd_sc[:, b, :],
                            )
                        dst = pt_sb[0:mw, mt, 4 * bg:4 * (bg + 1), :]
                        if (mt + bg) % 3 == 2:
                            nc.scalar.copy(out=dst, in_=ps[0:mw, 0:4 * QTW])
                        else:
                            nc.vector.tensor_copy(out=dst, in_=ps[0:mw, 0:4 * QTW])

                def av_proj(qt, pt_sb):
                    q0 = qt * QTW
                    softs.pop(qt)
                    o_sb = opool.tile([128, FT, QTW], dt.bfloat16, tag="o",
                                      name="o_sb")
                    for gp in range(H // 2):
                        ps = ps_b.tile([128, QTW], dt.float32, tag="b", name="ps")
                        for sub in range(2):
                            g = 2 * gp + sub
                            for mt in range(len(MT)):
                                mw = MT[mt]
                                nc.tensor.matmul(
                                    out=ps[64 * sub:64 * (sub + 1), :],
                                    lhsT=v_sb[0:mw, mt, 64 * g:64 * (g + 1)],
                                    rhs=pt_sb[0:mw, mt, :, NI * g:NI * (g + 1)],
                                    start=(mt == 0), stop=(mt == len(MT) - 1),
                                    skip_group_check=True,
                                )
                        if gp % 2 == 0:
                            nc.vector.tensor_copy(out=o_sb[:, gp, :], in_=ps[:])
                        else:
                            nc.scalar.copy(out=o_sb[:, gp, :], in_=ps[:])
                    ps = ps_a.tile([QTW, C], dt.float32, tag="a", name="ps")
                    for kc in range(FT):
                        for lo, hi in ((0, 512), (512, C)):
                            nc.tensor.matmul(
                                out=ps[:, lo:hi],
                                lhsT=o_sb[:, kc, :],
                                rhs=wp_sb[:, kc, lo:hi],
                                start=(kc == 0), stop=(kc == FT - 1),
                            )
                    out_sb = outp.tile([QTW, C], dt.float32, tag="out",
                                       name="out_sb")
                    nc.vector.tensor_tensor(
                        out=out_sb[:], in0=ps[:], in1=bias_sb[0:QTW, :],
                        op=mybir.AluOpType.add,
                    )
                    rows = min(N - q0, QTW)
                    nc.scalar.dma_start(out=y[bj, q0:q0 + rows, :],
                                        in_=out_sb[0:rows, :])

                # prologue as a closure queue: qkv chunks, then 3 staged
                # logits qtiles, then the first two pack reads
                pre = []
                for ft in range(FT):
                    for dst, w_sb, ntok in ((q_sb, wq_sb, NPAD), (k_sb, wk_sb, N)):
                        pre.append(lambda ft=ft, dst=dst, w_sb=w_sb, ntok=ntok:
                                   qk_chunk(ft, dst, w_sb, ntok))
                for qt in range(3):
                    pre.append(lambda qt=qt: new_lnat(qt))
                    for hp in range(H // 2):
                        pre.append(lambda qt=qt, hp=hp: logits_pair(qt, hp))
                    pre.append(lambda qt=qt: pack_writes(qt))
                    if qt < 2:
                        pre.append(lambda qt=qt: pack_read(qt))
                for mt in range(len(MT)):
                    pre.append(lambda mt=mt: v_chunk(mt))

                ctx.update(pre=pre, new_lnat=new_lnat, logits_pair=logits_pair,
                           pack_writes=pack_writes, pack_read=pack_read,
                           new_soft=new_soft, soft_block=soft_block,
                           postmix_group=postmix_group, av_proj=av_proj,
                           lpks=lpks, softs=softs)
                return ctx

            staged = {}
            for bi in range(BPC):
                ctx = staged.pop(bi) if bi in staged else make_batch(bi)
                while ctx["pre"]:
                    ctx["pre"].pop(0)()
                spare = []
                # iteration it: soft(it+1) | logits(it+4) | tail(it)
                for it in range(-1, QT):
                    sq = it + 1 if it + 1 < QT else None
                    lq = it + 4 if it + 4 < QT else None
                    tq = it if it >= 0 else None
                    if lq is None and bi + 1 < BPC and (bi + 1) not in staged:
                        staged[bi + 1] = make_batch(bi + 1)
                        spare = staged[bi + 1]["pre"]
                    if it >= 0 and it + 2 < QT:
                        ctx["pack_read"](it + 2)
                    if sq is not None:
                        ctx["new_soft"](sq)
                    if lq is not None:
                        ctx["new_lnat"](lq)
                    pt_sb = (ptp.tile([128, len(MT), BPQ, QTW], dt.bfloat16,
                                      tag="pt", name="pt_sb")
                             if tq is not None else None)
                    for bg in range(3):
                        if lq is not None:
                            ctx["logits_pair"](lq, 2 * bg)
                        elif spare:
                            spare.pop(0)()
                        if sq is not None:
                            ctx["soft_block"](sq, 4 * bg)
                            ctx["soft_block"](sq, 4 * bg + 1)
                        if tq is not None:
                            ctx["postmix_group"](tq, pt_sb, bg, (0, 1))
                        if sq is not None:
                            ctx["soft_block"](sq, 4 * bg + 2)
                        if tq is not None:
                            ctx["postmix_group"](tq, pt_sb, bg, (2, 3))
                        if sq is not None:
                            ctx["soft_block"](sq, 4 * bg + 3)
                        if tq is not None:
                            ctx["postmix_group"](tq, pt_sb, bg, (4,))
                        if lq is not None:
                            ctx["logits_pair"](lq, 2 * bg + 1)
                        elif spare:
                            spare.pop(0)()
                            if spare:
                                spare.pop(0)()
                    if sq is not None:
                        ctx["lpks"].pop(sq)
                    if lq is not None:
                        ctx["pack_writes"](lq)
                    if tq is not None:
                        ctx["av_proj"](tq, pt_sb)
    nc.compile()
    return nc


def _host_prep(x, Wqkv, Wproj, bproj, Wpre, Wpost):
    scale = D ** -0.5
    Wq = (Wqkv[0:C] * scale).T        # [C, C] lhsT for q (scale folded)
    Wk = Wqkv[C:2 * C].T
    Wv = Wqkv[2 * C:3 * C].T
    Wp = Wproj.T
    # h-major packed-block mixing matrices (p = 10*h + n_i)
    eye = np.eye(NI, dtype=np.float32)
    # bdpre[(10h+ni), (10g+nj)] = Wpre[g, h] * (ni == nj)
    bdpre = np.einsum("gh,ij->higj", Wpre.astype(np.float32), eye).reshape(QTW, QTW)
    # bdpostT[(10g+ni), (10g'+nj)] = Wpost[g', g] * (ni == nj)
    bdpostT = np.einsum("pg,ij->gipj", Wpost.astype(np.float32), eye).reshape(QTW, QTW)

    xT = np.zeros((B, C, NPAD), dtype=BF16)
    xT[:, :, 0:N] = np.ascontiguousarray(x.transpose(0, 2, 1)).astype(BF16)
    return {
        "xT": xT,
        "wqT": np.ascontiguousarray(Wq).astype(BF16),
        "wkT": np.ascontiguousarray(Wk).astype(BF16),
        "wvT": np.ascontiguousarray(Wv).astype(BF16),
        "wpT": np.ascontiguousarray(Wp).astype(BF16),
        "bdpre": bdpre.astype(BF16),
        "bdpostT": bdpostT.astype(BF16),
        "bias": bproj.astype(np.float32),
    }


def kernel(x, Wqkv, Wproj, bproj, Wpre, Wpost):
    x = np.asarray(x, dtype=np.float32)
    Wqkv = np.asarray(Wqkv, dtype=np.float32)
    Wproj = np.asarray(Wproj, dtype=np.float32)
    bproj = np.asarray(bproj, dtype=np.float32)
    Wpre = np.asarray(Wpre, dtype=np.float32)
    Wpost = np.asarray(Wpost, dtype=np.float32)

    host = _host_prep(x, Wqkv, Wproj, bproj, Wpre, Wpost)
    if "nc" not in _NC_CACHE:
        _NC_CACHE["nc"] = _build_nc()
    nc = _NC_CACHE["nc"]

    shared = {k: host[k] for k in
              ("wqT", "wkT", "wvT", "wpT", "bdpre", "bdpostT", "bias")}
    in_maps = []
    for core in range(NCORES):
        m = dict(shared)
        m["xT"] = host["xT"][core * BPC:(core + 1) * BPC]
        in_maps.append(m)

    res = run_bass_kernel_spmd(nc, in_maps, core_ids=list(range(NCORES)))
    out = np.concatenate([np.asarray(r["y"]) for r in res.results], axis=0)
    return out.astype(np.float32)



# revision 1
# speedup vs baseline: 1.3726x; 1.3726x over previous
"""Talking-heads attention (ViT-B/16-ish shapes) on 8 Trainium2 NeuronCores.

Problem: B=16, N=577, C=768, H=12 heads, d=64.
  qkv = x @ Wqkv.T ; logits = q k^T * scale ; pre-softmax head mix (Wpre);
  softmax ; post-softmax head mix (Wpost) ; out = (attn @ v) @ Wproj.T + b.

Distribution: pure data-parallel over batch, 2 batches per core, no
collectives.

Per-core design (all matmuls bf16 inputs, fp32 PSUM accumulation):
  - host pre-transposes x to [C, N] and pre-casts/packs all weights.
  - qkv:   q,k in [feat, tok] layout; v in [tok, feat] layout.
  - logits per head, K=64, two heads run concurrently via PE row groups.
  - talking-heads mixing runs as 120x120 block-diagonal matmuls in a packed
    layout [(h-major: p = 10h + n_i), m] over blocks of 10 query rows.
    The partition interleave that builds this layout is impossible in one
    SBUF->SBUF DMA (only the first AP dim may cross partitions), so the pack
    round-trips through a DRAM scratch laid out [p][b][m] per qtile:
    10 stride-10-partition write DMAs + 3 contiguous read DMAs. The read
    is detached from the logits stage (issued a full iteration later), so
    the DRAM bounce decouples producer and consumer completely.
  - softmax without max-subtraction (logits are small); exp on ScalarE with
    accum_out producing the row sums. The exp stream (~0.92us per block) is
    the slowest per-block stage, so the PE emission interleaves premix
    blocks with logits head-pairs and postmix groups of the two adjacent
    pipeline stages -- the in-order PE always has work while PSUM slots
    wait on exps.
  - normalization is folded into the postmix: rows of the moving operand
    bdpostT are scaled by 1/S per block (12 ops of [120,120] instead of
    [120,577]); exact since postmix contracts over p=(g,ni) and S is per-p.
  - post-mix is fused with the transpose AV needs: E-tile stationary,
    scaled block-diag Wpost^T moving, giving P'^T[m, (10g+n)] in PSUM.
  - AV consumes P'^T with a strided free AP per head; head pairs run
    concurrently via PE column groups; output feeds proj with no transpose.
  - Steady-state iteration `it` emits, interleaved per 4-block group:
      premix+exp(it+1) | logits(it+3) | postmix(it); then AV+proj(it),
      pack-writes(it+3); pack-read(it+2) opens the iteration.
"""

import numpy as np
import ml_dtypes

import concourse.bass as bass
import concourse.mybir as mybir
from concourse import bacc
from concourse.tile import TileContext
from concourse.bass_utils import run_bass_kernel_spmd

BF16 = ml_dtypes.bfloat16

B, N, C, H = 16, 577, 768, 12
D = C // H                 # 64
NCORES = 8
BPC = B // NCORES          # batches per core = 2
NPAD = 600                 # padded query-token count (5 qtiles of 120)
QT = 5                     # query tiles
QTW = 120                  # rows per query tile
NI = 10                    # query rows per packed block
BPQ = QTW // NI            # blocks per qtile = 12
FT = C // 128              # feature tiles = 6
MT = [128, 128, 128, 128, 65]   # key-token tiles (sum 577)
MOF = [0, 128, 256, 384, 512]

_NC_CACHE = {}


def _build_nc(debug=False):
    nc = bacc.Bacc("TRN2", target_bir_lowering=False)
    dt = mybir.dt

    xT = nc.dram_tensor("xT", [BPC, C, NPAD], dt.bfloat16, kind="ExternalInput")
    wqT = nc.dram_tensor("wqT", [C, C], dt.bfloat16, kind="ExternalInput")
    wkT = nc.dram_tensor("wkT", [C, C], dt.bfloat16, kind="ExternalInput")
    wvT = nc.dram_tensor("wvT", [C, C], dt.bfloat16, kind="ExternalInput")
    wpT = nc.dram_tensor("wpT", [C, C], dt.bfloat16, kind="ExternalInput")
    bdpre = nc.dram_tensor("bdpre", [QTW, QTW], dt.bfloat16, kind="ExternalInput")
    bdpostT = nc.dram_tensor("bdpostT", [QTW, QTW], dt.bfloat16, kind="ExternalInput")
    bias = nc.dram_tensor("bias", [C], dt.float32, kind="ExternalInput")
    y = nc.dram_tensor("y", [BPC, N, C], dt.float32, kind="ExternalOutput")
    # logits bounce scratch in natural order [batch][qtile][nq][h][m];
    # the packed interleave happens on the strided read side
    pk = nc.dram_tensor("pk", [BPC, QT, QTW, H, N], dt.bfloat16, kind="Internal")

    with TileContext(nc) as tc:
        with (
            tc.tile_pool(name="consts", bufs=1) as consts,
            tc.tile_pool(name="xp", bufs=1) as xp,
            tc.tile_pool(name="qkv", bufs=2) as qkvp,
            tc.tile_pool(name="vpool", bufs=1) as vpool,
            tc.tile_pool(name="opool", bufs=2) as opool,
            tc.tile_pool(name="lnatp", bufs=2) as lnatp,
            tc.tile_pool(name="lpkp", bufs=2) as lpkp,
            tc.tile_pool(name="ep", bufs=2) as ep,
            tc.tile_pool(name="ptp", bufs=1) as ptp,
            tc.tile_pool(name="bdp", bufs=2) as bdp,
            tc.tile_pool(name="stage", bufs=2) as stage,
            tc.tile_pool(name="outp", bufs=2) as outp,
            tc.tile_pool(name="ps_a", bufs=2, space="PSUM") as ps_a,
            tc.tile_pool(name="ps_b", bufs=4, space="PSUM") as ps_b,
        ):
            # ---- constants (split per kc-column so qkv starts early) ----
            wq_sb = consts.tile([128, FT, C], dt.bfloat16, tag="wq")
            wk_sb = consts.tile([128, FT, C], dt.bfloat16, tag="wk")
            wv_sb = consts.tile([128, FT, C], dt.bfloat16, tag="wv")
            wp_sb = consts.tile([128, FT, C], dt.bfloat16, tag="wp")
            for w_sb, w_dr, eng in ((wq_sb, wqT, nc.scalar), (wk_sb, wkT, nc.gpsimd),
                                    (wv_sb, wvT, nc.scalar), (wp_sb, wpT, nc.gpsimd)):
                eng.dma_start(out=w_sb[:], in_=w_dr.rearrange("(t p) f -> p t f", p=128))
            bdpre_sb = consts.tile([QTW, QTW], dt.bfloat16, tag="bdpre")
            nc.scalar.dma_start(out=bdpre_sb[:], in_=bdpre[:])
            bdpostT_sb = consts.tile([QTW, QTW], dt.bfloat16, tag="bdpostT")
            nc.gpsimd.dma_start(out=bdpostT_sb[:], in_=bdpostT[:])
            bias_sb = consts.tile([128, C], dt.float32, tag="bias")
            nc.scalar.dma_start(
                out=bias_sb[:],
                in_=bass.AP(tensor=bias[:].tensor, offset=0, ap=[[0, 128], [1, C]]),
            )

            def make_batch(bj):
                """Per-batch pipeline context; all emitters bound to batch bj."""
                ctx = {}
                lnats, lpks, softs = {}, {}, {}

                xT_sb = xp.tile([128, FT, NPAD], dt.bfloat16, tag="xT", name="xT_sb")
                nc.sync.dma_start(
                    out=xT_sb[:], in_=xT[bj].rearrange("(t p) n -> p t n", p=128)
                )
                q_sb = qkvp.tile([128, FT, NPAD], dt.bfloat16, tag="q", name="q_sb")
                k_sb = qkvp.tile([128, FT, N], dt.bfloat16, tag="k", name="k_sb")
                v_sb = vpool.tile([128, len(MT), C], dt.bfloat16, tag="v", name="v_sb")
                ctx["v_sb"] = v_sb

                def qk_chunk(ft, dst, w_sb, ntok):
                    ps = ps_a.tile([128, NPAD], dt.float32, tag="a", name="ps")
                    for kc in range(FT):
                        for lo, hi in ((0, 512), (512, ntok)):
                            nc.tensor.matmul(
                                out=ps[:, lo:hi],
                                lhsT=w_sb[:, kc, ft * 128:(ft + 1) * 128],
                                rhs=xT_sb[:, kc, lo:hi],
                                start=(kc == 0), stop=(kc == FT - 1),
                            )
                    if ft % 2 == 0:
                        nc.vector.tensor_copy(out=dst[:, ft, :], in_=ps[:, 0:ntok])
                    else:
                        nc.scalar.copy(out=dst[:, ft, :], in_=ps[:, 0:ntok])

                def v_chunk(mt):
                    ps = ps_a.tile([128, C], dt.float32, tag="a", name="ps")
                    mw = MT[mt]
                    for kc in range(FT):
                        for lo, hi in ((0, 512), (512, C)):
                            nc.tensor.matmul(
                                out=ps[0:mw, lo:hi],
                                lhsT=xT_sb[:, kc, MOF[mt]:MOF[mt] + mw],
                                rhs=wv_sb[:, kc, lo:hi],
                                start=(kc == 0), stop=(kc == FT - 1),
                            )
                    if mt % 2 == 0:
                        nc.vector.tensor_copy(out=v_sb[0:mw, mt, :], in_=ps[0:mw, 0:C])
                    else:
                        nc.scalar.copy(out=v_sb[0:mw, mt, :], in_=ps[0:mw, 0:C])

                def new_lnat(qt):
                    lnats[qt] = lnatp.tile([QTW, H, N], dt.bfloat16, tag="lnat",
                                           name="l_nat")

                def logits_pair(qt, hp):
                    q0 = qt * QTW
                    l_nat = lnats[qt]
                    ps0 = ps_a.tile([QTW, N], dt.float32, tag="a", name="ps0")
                    ps1 = ps_a.tile([QTW, N], dt.float32, tag="a", name="ps1")
                    for sub, ps in ((0, ps0), (1, ps1)):
                        pbase = 64 * sub
                        for lo, hi in ((0, 512), (512, N)):
                            nc.tensor.matmul(
                                out=ps[:, lo:hi],
                                lhsT=q_sb[pbase:pbase + 64, hp, q0:q0 + QTW],
                                rhs=k_sb[pbase:pbase + 64, hp, lo:hi],
                            )
                    if hp in (2, 5):
                        nc.scalar.copy(out=l_nat[:, 2 * hp, :], in_=ps0[:])
                    else:
                        nc.vector.tensor_copy(out=l_nat[:, 2 * hp, :], in_=ps0[:])
                    nc.vector.tensor_copy(out=l_nat[:, 2 * hp + 1, :], in_=ps1[:])

                def pack_writes(qt):
                    nc.sync.dma_start(out=pk[bj, qt], in_=lnats.pop(qt)[:])

                def pack_read(qt):
                    l_pk = lpkp.tile([QTW, BPQ, N], dt.bfloat16, tag="lpk",
                                     name="l_pk")
                    ov = l_pk[:].rearrange("(h n) b m -> n h b m", n=NI)
                    iv = pk[bj, qt].rearrange("(b n) h m -> n h b m", n=NI)
                    for ni in range(NI):
                        nc.gpsimd.dma_start(out=ov[ni], in_=iv[ni])
                    lpks[qt] = l_pk

                def new_soft(qt):
                    softs[qt] = (
                        ep.tile([QTW, BPQ, N], dt.bfloat16, tag="e", name="e_sb"),
                        stage.tile([QTW, BPQ], dt.float32, tag="s", name="s_sb"),
                        stage.tile([QTW, BPQ], dt.float32, tag="sinv", name="sinv"),
                        bdp.tile([QTW, BPQ, QTW], dt.bfloat16, tag="bdsc",
                                 name="bd_sc"),
                    )

                def soft_block(qt, b):
                    e_sb, s_sb, sinv, bd_sc = softs[qt]
                    l_pk = lpks[qt]
                    ps = ps_a.tile([QTW, N], dt.float32, tag="a", name="ps")
                    for lo, hi in ((0, 512), (512, N)):
                        nc.tensor.matmul(
                            out=ps[:, lo:hi], lhsT=bdpre_sb[:], rhs=l_pk[:, b, lo:hi]
                        )
                    nc.scalar.activation(
                        out=e_sb[:, b, :], in_=ps[:],
                        func=mybir.ActivationFunctionType.Exp,
                        accum_out=s_sb[:, b:b + 1],
                    )
                    nc.vector.reciprocal(out=sinv[:, b:b + 1], in_=s_sb[:, b:b + 1])
                    nc.vector.tensor_scalar_mul(
                        bd_sc[:, b, :], bdpostT_sb[:], sinv[:, b:b + 1]
                    )

                def postmix_group(qt, pt_sb, bg, mts):
                    e_sb, _, _, bd_sc = softs[qt]
                    for mt in mts:
                        mw = MT[mt]
                        ps = ps_b.tile([128, 4 * QTW], dt.float32, tag="b", name="ps")
                        for sl in range(4):
                            b = 4 * bg + sl
                            nc.tensor.matmul(
                                out=ps[0:mw, sl * QTW:(sl + 1) * QTW],
                                lhsT=e_sb[:, b, MOF[mt]:MOF[mt] + mw],
                                rhs=bd_sc[:, b, :],
                            )
                        dst = pt_sb[0:mw, mt, 4 * bg:4 * (bg + 1), :]
                        if (mt + bg) % 3 == 2:
                            nc.scalar.copy(out=dst, in_=ps[0:mw, 0:4 * QTW])
                        else:
                            nc.vector.tensor_copy(out=dst, in_=ps[0:mw, 0:4 * QTW])

                def av_proj(qt, pt_sb):
                    q0 = qt * QTW
                    softs.pop(qt)
                    o_sb = opool.tile([128, FT, QTW], dt.bfloat16, tag="o",
                                      name="o_sb")
                    for gp in range(H // 2):
                        ps = ps_b.tile([128, QTW], dt.float32, tag="b", name="ps")
                        for sub in range(2):
                            g = 2 * gp + sub
                            for mt in range(len(MT)):
                                mw = MT[mt]
                                nc.tensor.matmul(
                                    out=ps[64 * sub:64 * (sub + 1), :],
                                    lhsT=v_sb[0:mw, mt, 64 * g:64 * (g + 1)],
                                    rhs=pt_sb[0:mw, mt, :, NI * g:NI * (g + 1)],
                                    start=(mt == 0), stop=(mt == len(MT) - 1),
                                    skip_group_check=True,
                                )
                        if gp % 2 == 0:
                            nc.vector.tensor_copy(out=o_sb[:, gp, :], in_=ps[:])
                        else:
                            nc.scalar.copy(out=o_sb[:, gp, :], in_=ps[:])
                    ps = ps_a.tile([QTW, C], dt.float32, tag="a", name="ps")
                    for kc in range(FT):
                        for lo, hi in ((0, 512), (512, C)):
                            nc.tensor.matmul(
                                out=ps[:, lo:hi],
                                lhsT=o_sb[:, kc, :],
                                rhs=wp_sb[:, kc, lo:hi],
                                start=(kc == 0), stop=(kc == FT - 1),
                            )
                    out_sb = outp.tile([QTW, C], dt.float32, tag="out",
                                       name="out_sb")
                    nc.vector.tensor_tensor(
                        out=out_sb[:], in0=ps[:], in1=bias_sb[0:QTW, :],
                        op=mybir.AluOpType.add,
                    )
                    rows = min(N - q0, QTW)
                    nc.scalar.dma_start(out=y[bj, q0:q0 + rows, :],
                                        in_=out_sb[0:rows, :])

                # prologue as a closure queue: qkv chunks, then 3 staged
                # logits qtiles, then the first two pack reads
                pre = []
                for ft in range(FT):
                    for dst, w_sb, ntok in ((q_sb, wq_sb, NPAD), (k_sb, wk_sb, N)):
                        pre.append(lambda ft=ft, dst=dst, w_sb=w_sb, ntok=ntok:
                                   qk_chunk(ft, dst, w_sb, ntok))
                for qt in range(3):
                    pre.append(lambda qt=qt: new_lnat(qt))
                    for hp in range(H // 2):
                        pre.append(lambda qt=qt, hp=hp: logits_pair(qt, hp))
                    pre.append(lambda qt=qt: pack_writes(qt))
                    if qt < 2:
                        pre.append(lambda qt=qt: pack_read(qt))
                for mt in range(len(MT)):
                    pre.append(lambda mt=mt: v_chunk(mt))

                ctx.update(pre=pre, new_lnat=new_lnat, logits_pair=logits_pair,
                           pack_writes=pack_writes, pack_read=pack_read,
                           new_soft=new_soft, soft_block=soft_block,
                           postmix_group=postmix_group, av_proj=av_proj,
                           lpks=lpks, softs=softs)
                return ctx

            staged = {}
            for bi in range(BPC):
                ctx = staged.pop(bi) if bi in staged else make_batch(bi)
                while ctx["pre"]:
                    ctx["pre"].pop(0)()
                spare = []
                # iteration it: soft(it+1) | logits(it+4) | tail(it)
                for it in range(-1, QT):
                    sq = it + 1 if it + 1 < QT else None
                    lq = it + 4 if it + 4 < QT else None
                    tq = it if it >= 0 else None
                    if lq is None and bi + 1 < BPC and (bi + 1) not in staged:
                        staged[bi + 1] = make_batch(bi + 1)
                        spare = staged[bi + 1]["pre"]
                    if it >= 0 and it + 2 < QT:
                        ctx["pack_read"](it + 2)
                    if sq is not None:
                        ctx["new_soft"](sq)
                    if lq is not None:
                        ctx["new_lnat"](lq)
                    pt_sb = (ptp.tile([128, len(MT), BPQ, QTW], dt.bfloat16,
                                      tag="pt", name="pt_sb")
                             if tq is not None else None)
                    for bg in range(3):
                        if lq is not None:
                            ctx["logits_pair"](lq, 2 * bg)
                        elif spare:
                            spare.pop(0)()
                        if sq is not None:
                            ctx["soft_block"](sq, 4 * bg)
                            ctx["soft_block"](sq, 4 * bg + 1)
                        if tq is not None:
                            ctx["postmix_group"](tq, pt_sb, bg, (0, 1))
                        if sq is not None:
                            ctx["soft_block"](sq, 4 * bg + 2)
                        if tq is not None:
                            ctx["postmix_group"](tq, pt_sb, bg, (2, 3))
                        if sq is not None:
                            ctx["soft_block"](sq, 4 * bg + 3)
                        if tq is not None:
                            ctx["postmix_group"](tq, pt_sb, bg, (4,))
                        if lq is not None:
                            ctx["logits_pair"](lq, 2 * bg + 1)
                        elif spare:
                            spare.pop(0)()
                            if spare:
                                spare.pop(0)()
                    if sq is not None:
                        ctx["lpks"].pop(sq)
                    if lq is not None:
                        ctx["pack_writes"](lq)
                    if tq is not None:
                        ctx["av_proj"](tq, pt_sb)
    nc.compile()
    return nc


def _host_prep(x, Wqkv, Wproj, bproj, Wpre, Wpost):
    scale = D ** -0.5
    Wq = (Wqkv[0:C] * scale).T        # [C, C] lhsT for q (scale folded)
    Wk = Wqkv[C:2 * C].T
    Wv = Wqkv[2 * C:3 * C].T
    Wp = Wproj.T
    # h-major packed-block mixing matrices (p = 10*h + n_i)
    eye = np.eye(NI, dtype=np.float32)
    # bdpre[(10h+ni), (10g+nj)] = Wpre[g, h] * (ni == nj)
    bdpre = np.einsum("gh,ij->higj", Wpre.astype(np.float32), eye).reshape(QTW, QTW)
    # bdpostT[(10g+ni), (10g'+nj)] = Wpost[g', g] * (ni == nj)
    bdpostT = np.einsum("pg,ij->gipj", Wpost.astype(np.float32), eye).reshape(QTW, QTW)

    xT = np.zeros((B, C, NPAD), dtype=BF16)
    xT[:, :, 0:N] = np.ascontiguousarray(x.transpose(0, 2, 1)).astype(BF16)
    return {
        "xT": xT,
        "wqT": np.ascontiguousarray(Wq).astype(BF16),
        "wkT": np.ascontiguousarray(Wk).astype(BF16),
        "wvT": np.ascontiguousarray(Wv).astype(BF16),
        "wpT": np.ascontiguousarray(Wp).astype(BF16),
        "bdpre": bdpre.astype(BF16),
        "bdpostT": bdpostT.astype(BF16),
        "bias": bproj.astype(np.float32),
    }


def kernel(x, Wqkv, Wproj, bproj, Wpre, Wpost):
    x = np.asarray(x, dtype=np.float32)
    Wqkv = np.asarray(Wqkv, dtype=np.float32)
    Wproj = np.asarray(Wproj, dtype=np.float32)
    bproj = np.asarray(bproj, dtype=np.float32)
    Wpre = np.asarray(Wpre, dtype=np.float32)
    Wpost = np.asarray(Wpost, dtype=np.float32)

    host = _host_prep(x, Wqkv, Wproj, bproj, Wpre, Wpost)
    if "nc" not in _NC_CACHE:
        _NC_CACHE["nc"] = _build_nc()
    nc = _NC_CACHE["nc"]

    shared = {k: host[k] for k in
              ("wqT", "wkT", "wvT", "wpT", "bdpre", "bdpostT", "bias")}
    in_maps = []
    for core in range(NCORES):
        m = dict(shared)
        m["xT"] = host["xT"][core * BPC:(core + 1) * BPC]
        in_maps.append(m)

    res = run_bass_kernel_spmd(nc, in_maps, core_ids=list(range(NCORES)))
    out = np.concatenate([np.asarray(r["y"]) for r in res.results], axis=0)
    return out.astype(np.float32)

